# revision 18
# baseline (speedup 1.0000x reference)
"""Bass/Trainium2 kernel for nn_GatherUpdate: LayerNorm + Linear + per-atom
row gather + residual add, data-parallel over batch across 8 NeuronCores.

reference:
    normed = LayerNorm(s) * gamma + beta            # s: [B, 2048, 384]
    upd    = normed @ W.T                           # W: [128, 384] -> [B, 2048, 128]
    out    = atom_embed + upd[:, cond_to_s_idx, :]  # atom_embed: [B, 32768, 128]

Per-core plan (core b handles batch b):
  1. LN over 16 tiles of [128 res, 384] via bn_stats/bn_aggr.
  2. PE-transpose normed tiles (cs onto partitions), matmul against
     gamma-folded W^T chunks, accumulate in PSUM; beta folded in via a K=1
     ones-row matmul against (beta @ W.T).
  3. Write upd table [2048, 128] to DRAM scratch.
  4. dma_gather 512B rows from the table (32 calls x 1024 idxs — the HW
     SWDGE ring holds ~127 descriptors/engine — rotated over 4 SWDGE
     queues), add to atom_embed tiles, store. Atom tiles use contiguous
     2MB DMAs; the host pre-permutes the int16 index stream so the gather
     output layout matches the atom tiles.
"""

import sys

sys.path.insert(0, "/opt/trn_rl_repo")

import numpy as np

B = 8
N_ATOMS = 32768
N_RES = 2048
C_S = 384
C_ATOM = 128
EPS = 1e-5
P = 128
N_RES_TILES = N_RES // P  # 16
N_CHUNKS = 8  # atom chunks per core
CHUNK = N_ATOMS // N_CHUNKS  # 4096 atoms
CT = CHUNK // P  # 32 tiles of 128 atoms per chunk
KC = C_S // P  # 3 contraction chunks
GSUB = 1024  # idxs per dma_gather (HW SWDGE ring holds ~127 desc/engine)
GQ = CHUNK // GSUB  # 4 sub-gathers per atom chunk
GT = GSUB // P  # 8 x 128-atom tiles per sub-gather
NQUEUES = 4  # rotate SWDGE queues so desc-gen overlaps in-flight gathers

_compiled = None


def _build(repeat=1):
    """Build the per-core program. repeat>1 unrolls the whole pipeline N
    times (used only for timing: wall(N)-wall(1) cancels dispatch/transfer
    overhead)."""
    import concourse.bacc as bacc
    import concourse.tile as tile
    from concourse import mybir
    from concourse.masks import make_identity
    from concourse.tile import add_dep_helper

    f32 = mybir.dt.float32

    nc = bacc.Bacc(
        "TRN2", target_bir_lowering=False, debug=False, num_swdge_queues=NQUEUES
    )

    atom = nc.dram_tensor("atom", [N_ATOMS, C_ATOM], f32, kind="ExternalInput")
    s_in = nc.dram_tensor("s_in", [N_RES, C_S], f32, kind="ExternalInput")
    idx16 = nc.dram_tensor(
        "idx16", [P, N_ATOMS // 16], mybir.dt.int16, kind="ExternalInput"
    )
    wg = nc.dram_tensor("wg", [P, C_S], f32, kind="ExternalInput")
    wt = nc.dram_tensor("wt", [P, C_S], f32, kind="ExternalInput")
    beta3 = nc.dram_tensor("beta3", [P, KC], f32, kind="ExternalInput")
    out = nc.dram_tensor("out", [N_ATOMS, C_ATOM], f32, kind="ExternalOutput")
    upd_dram = nc.dram_tensor("upd_scratch", [N_RES, C_ATOM], f32, kind="Internal")

    with tile.TileContext(nc) as tc:
        with (
            tc.tile_pool(name="singles", bufs=1) as singles,
            tc.tile_pool(name="stats", bufs=4) as stats,
            tc.tile_pool(name="t2p", bufs=3) as t2p,
            tc.tile_pool(name="t2tp", bufs=6) as t2tp,
            tc.tile_pool(name="psum_tt", bufs=4, space="PSUM") as psum_tt,
            tc.tile_pool(name="psum_mm", bufs=2, space="PSUM") as psum_mm,
            tc.tile_pool(name="atoms", bufs=4) as atoms_pool,
            tc.tile_pool(name="gath", bufs=4) as gath_pool,
        ):
            # --- constants / small inputs ---
            idx_sb = singles.tile([P, N_ATOMS // 16], mybir.dt.int16)
            nc.sync.dma_start(out=idx_sb[:], in_=idx16.ap())
            wg_sb = singles.tile([P, C_S], f32)
            nc.sync.dma_start(out=wg_sb[:], in_=wg.ap())
            wt_sb = singles.tile([P, C_S], f32)
            nc.sync.dma_start(out=wt_sb[:], in_=wt.ap())
            beta_sb = singles.tile([P, KC], f32)
            nc.sync.dma_start(out=beta_sb[:], in_=beta3.ap())

            ident = singles.tile([P, P], f32)
            make_identity(nc, ident[:])
            ones1 = singles.tile([1, P], f32)
            nc.vector.memset(ones1[:], 1.0)
            eps_t = singles.tile([P, 1], f32)
            nc.vector.memset(eps_t[:], EPS)

            # --- bias row: (beta @ W.T) [1, C_ATOM] ---
            bias_ps = psum_mm.tile([1, C_ATOM], f32, tag="biasps")
            for k in range(KC):
                nc.tensor.matmul(
                    bias_ps[:],
                    lhsT=beta_sb[:, k : k + 1],
                    rhs=wt_sb[:, k * P : (k + 1) * P],
                    start=(k == 0),
                    stop=(k == KC - 1),
                )
            bias_sb = singles.tile([1, C_ATOM], f32)
            nc.vector.tensor_copy(out=bias_sb[:], in_=bias_ps[:])

            prev_gathers = []
            for _rep in range(repeat):
                # --- load s p-major: s_big[p, j, :] = row p*16 + j, so the
                # load (and the matching upd store) is contiguous per
                # partition. LN is per-row, so any row->lane mapping works.
                # Split into quarters so LN on tile 0 starts ~4x sooner.
                s_big = singles.tile([P, N_RES_TILES, C_S], f32, tag="s_big")
                s_pt = s_in.ap().rearrange("(p t) c -> p t c", p=P)
                for sq in range(4):
                    tq = N_RES_TILES // 4
                    nc.sync.dma_start(
                        out=s_big[:, sq * tq : (sq + 1) * tq, :],
                        in_=s_pt[:, sq * tq : (sq + 1) * tq, :],
                    )

                # --- LN + matmul into upd table ---
                upd_big = singles.tile([P, N_RES_TILES, C_ATOM], f32, tag="upd_big")
                for i in range(N_RES_TILES):
                    st6 = stats.tile([P, 6], f32)
                    nc.vector.bn_stats(out=st6[:], in_=s_big[:, i, :])
                    mv = stats.tile([P, 2], f32)
                    nc.vector.bn_aggr(out=mv[:], in_=st6[:])
                    std = stats.tile([P, 1], f32)
                    nc.scalar.activation(
                        out=std[:],
                        in_=mv[:, 1:2],
                        func=mybir.ActivationFunctionType.Sqrt,
                        bias=eps_t[:],
                    )
                    rstd = stats.tile([P, 1], f32)
                    nc.vector.reciprocal(out=rstd[:], in_=std[:])
                    t2 = t2p.tile([P, C_S], f32)
                    nc.vector.tensor_scalar(
                        out=t2[:],
                        in0=s_big[:, i, :],
                        scalar1=mv[:, 0:1],
                        scalar2=rstd[:],
                        op0=mybir.AluOpType.subtract,
                        op1=mybir.AluOpType.mult,
                    )
                    t2t_chunks = []
                    for k in range(KC):
                        tt_ps = psum_tt.tile([P, P], f32)
                        nc.tensor.transpose(
                            out=tt_ps[:],
                            in_=t2[:, k * P : (k + 1) * P],
                            identity=ident[:],
                        )
                        t2t = t2tp.tile([P, P], f32)
                        nc.vector.tensor_copy(out=t2t[:], in_=tt_ps[:])
                        t2t_chunks.append(t2t)
                    upd_ps = psum_mm.tile([P, C_ATOM], f32, tag="updps")
                    for k in range(KC):
                        nc.tensor.matmul(
                            upd_ps[:],
                            lhsT=t2t_chunks[k][:],
                            rhs=wg_sb[:, k * P : (k + 1) * P],
                            start=(k == 0),
                            stop=False,
                        )
                    nc.tensor.matmul(
                        upd_ps[:], lhsT=ones1[:], rhs=bias_sb[:], start=False, stop=True
                    )
                    nc.vector.tensor_copy(out=upd_big[:, i, :], in_=upd_ps[:])

                # store table in quarters so stores overlap later LN tiles;
                # gathers gate on all four. p-major: upd_big[p, j, :] is
                # table row p*16 + j -> contiguous per partition.
                upd_stores = []
                upd_pt = upd_dram.ap().rearrange("(p t) c -> p t c", p=P)
                for sq in range(4):
                    tq = N_RES_TILES // 4
                    us = nc.sync.dma_start(
                        out=upd_pt[:, sq * tq : (sq + 1) * tq, :],
                        in_=upd_big[:, sq * tq : (sq + 1) * tq, :],
                    )
                    upd_stores.append(us)
                # WAR: don't overwrite the table while last rep still gathers
                for pg in prev_gathers:
                    for us in upd_stores:
                        add_dep_helper(
                            us.ins, pg.ins, reason="WAR on upd table across reps"
                        )
                prev_gathers = []

                # --- gather + residual add over 8 chunks of 4096 atoms ---
                for c in range(N_CHUNKS):
                    at = atoms_pool.tile([P, CT, C_ATOM], f32, tag="at")
                    nc.sync.dma_start(
                        out=at[:],
                        in_=atom.ap()[c * CHUNK : (c + 1) * CHUNK, :].rearrange(
                            "(p t) c -> p t c", p=P
                        ),
                    )
                    g = gath_pool.tile([P, CT, C_ATOM], f32, tag="g")
                    for q in range(GQ):
                        gi = nc.gpsimd.dma_gather(
                            g[:, q * GT : (q + 1) * GT, :],
                            upd_dram.ap(),
                            idx_sb[
                                :,
                                c * (CHUNK // 16) + q * (GSUB // 16) : c * (CHUNK // 16)
                                + (q + 1) * (GSUB // 16),
                            ],
                            GSUB,
                            GSUB,
                            C_ATOM,
                            queue_num=(c * GQ + q) % NQUEUES,
                        )
                        for us in upd_stores:
                            add_dep_helper(
                                gi.ins, us.ins, reason="upd table must be in DRAM"
                            )
                        prev_gathers.append(gi)
                    nc.vector.tensor_add(out=at[:], in0=at[:], in1=g[:])
                    nc.sync.dma_start(
                        out=out.ap()[c * CHUNK : (c + 1) * CHUNK, :].rearrange(
                            "(p t) c -> p t c", p=P
                        ),
                        in_=at[:],
                    )

    nc.compile()
    return nc


def _prep_core_inputs(atom_embed, s, cond_to_s_idx, ln_gamma, ln_beta, W):
    """Host-side sharding + layout marshalling (no math beyond folding the
    LN scale into the weight layout)."""
    # gamma-folded W^T, chunked so cs-chunk k sits at free columns [k*128, ...)
    wg_full = (W * ln_gamma[None, :]).T.astype(np.float32)  # [C_S, C_ATOM]
    wg_host = np.ascontiguousarray(
        wg_full.reshape(KC, P, C_ATOM).transpose(1, 0, 2).reshape(P, C_S)
    )
    wt_full = np.ascontiguousarray(W.T.astype(np.float32))  # [C_S, C_ATOM]
    wt_host = np.ascontiguousarray(
        wt_full.reshape(KC, P, C_ATOM).transpose(1, 0, 2).reshape(P, C_S)
    )
    beta_host = np.ascontiguousarray(
        ln_beta.astype(np.float32).reshape(KC, P).T
    )  # [P, KC]

    in_maps = []
    for b in range(B):
        idxb = cond_to_s_idx[b].astype(np.int16)  # values < 2048
        # atom tile layout within a 4096-chunk: at[p, t] = atom p*CT + t.
        # sub-gather q writes g[j%128, q*GT + j//128] for list position j, so
        # position j of sub-gather q must hold the index of atom
        # (j%128)*CT + q*GT + j//128.
        A = idxb.reshape(N_CHUNKS, P, GQ, GT)  # [c, p, q, u]
        L = A.transpose(0, 2, 3, 1).reshape(N_CHUNKS, GQ, GSUB)  # j = u*128+p
        # wrap within each sub-gather: position j -> [j%16, j//16]
        Wr = L.reshape(N_CHUNKS, GQ, GSUB // 16, 16).transpose(0, 1, 3, 2)
        idx_full = np.ascontiguousarray(
            Wr.transpose(2, 0, 1, 3).reshape(16, N_ATOMS // 16)
        )
        idx_rep = np.ascontiguousarray(np.tile(idx_full, (P // 16, 1)))
        in_maps.append(
            {
                "atom": np.ascontiguousarray(atom_embed[b]),
                "s_in": np.ascontiguousarray(s[b]),
                "idx16": idx_rep,
                "wg": wg_host,
                "wt": wt_host,
                "beta3": beta_host,
            }
        )
    return in_maps


def kernel(atom_embed, s, cond_to_s_idx, ln_gamma, ln_beta, W):
    global _compiled
    from concourse.bass_utils import run_bass_kernel_spmd

    atom_embed = np.asarray(atom_embed, dtype=np.float32)
    s = np.asarray(s, dtype=np.float32)
    cond_to_s_idx = np.asarray(cond_to_s_idx)
    ln_gamma = np.asarray(ln_gamma, dtype=np.float32)
    ln_beta = np.asarray(ln_beta, dtype=np.float32)
    W = np.asarray(W, dtype=np.float32)

    if _compiled is None:
        _compiled = _build()
    in_maps = _prep_core_inputs(atom_embed, s, cond_to_s_idx, ln_gamma, ln_beta, W)
    res = run_bass_kernel_spmd(_compiled, in_maps, core_ids=list(range(B)))
    out = np.stack([res.results[b]["out"] for b in range(B)], axis=0)
    return out


# revision 25
# speedup vs baseline: 1.2736x; 1.2736x over previous
"""Bass/Trainium2 kernel for nn_GatherUpdate: LayerNorm + Linear + per-atom
row gather + residual add, data-parallel over batch across 8 NeuronCores.

reference:
    normed = LayerNorm(s) * gamma + beta            # s: [B, 2048, 384]
    upd    = normed @ W.T                           # W: [128, 384] -> [B, 2048, 128]
    out    = atom_embed + upd[:, cond_to_s_idx, :]  # atom_embed: [B, 32768, 128]

Per-core plan (core b handles batch b):
  1. LN over 16 tiles of [128 res, 384] via bn_stats/bn_aggr.
  2. PE-transpose normed tiles (cs onto partitions), matmul against
     gamma-folded W^T chunks, accumulate in PSUM; beta folded in via a K=1
     ones-row matmul against (beta @ W.T).
  3. Write upd table [2048, 128] to DRAM scratch.
  4. dma_gather 512B rows from the table (32 calls x 1024 idxs — the HW
     SWDGE ring holds ~127 descriptors/engine — rotated over 4 SWDGE
     queues), add to atom_embed tiles, store. Atom tiles use contiguous
     2MB DMAs; the host pre-permutes the int16 index stream so the gather
     output layout matches the atom tiles.
"""

import sys

sys.path.insert(0, "/opt/trn_rl_repo")

import numpy as np

B = 8
N_ATOMS = 32768
N_RES = 2048
C_S = 384
C_ATOM = 128
EPS = 1e-5
P = 128
N_RES_TILES = N_RES // P  # 16
N_CHUNKS = 8  # atom chunks per core
CHUNK = N_ATOMS // N_CHUNKS  # 4096 atoms
CT = CHUNK // P  # 32 tiles of 128 atoms per chunk
KC = C_S // P  # 3 contraction chunks
GSUB = 1024  # idxs per dma_gather (HW SWDGE ring holds ~127 desc/engine)
GQ = CHUNK // GSUB  # 4 sub-gathers per atom chunk
GT = GSUB // P  # 8 x 128-atom tiles per sub-gather
NQUEUES = 4  # rotate SWDGE queues so desc-gen overlaps in-flight gathers

_compiled = None


def _build(repeat=1):
    """Build the per-core program. repeat>1 unrolls the whole pipeline N
    times (used only for timing: wall(N)-wall(1) cancels dispatch/transfer
    overhead)."""
    import concourse.bacc as bacc
    import concourse.tile as tile
    from concourse import mybir
    from concourse.masks import make_identity
    from concourse.tile import add_dep_helper

    f32 = mybir.dt.float32

    nc = bacc.Bacc(
        "TRN2", target_bir_lowering=False, debug=False, num_swdge_queues=NQUEUES
    )

    atom = nc.dram_tensor("atom", [N_ATOMS, C_ATOM], f32, kind="ExternalInput")
    s_in = nc.dram_tensor("s_in", [N_RES, C_S], f32, kind="ExternalInput")
    idx16 = nc.dram_tensor(
        "idx16", [P, N_ATOMS // 16], mybir.dt.int16, kind="ExternalInput"
    )
    wg = nc.dram_tensor("wg", [P, C_S], f32, kind="ExternalInput")
    wt = nc.dram_tensor("wt", [P, C_S], f32, kind="ExternalInput")
    beta3 = nc.dram_tensor("beta3", [P, KC], f32, kind="ExternalInput")
    out = nc.dram_tensor("out", [N_ATOMS, C_ATOM], f32, kind="ExternalOutput")
    upd_dram = nc.dram_tensor("upd_scratch", [N_RES, C_ATOM], f32, kind="Internal")

    with tile.TileContext(nc) as tc:
        with (
            tc.tile_pool(name="singles", bufs=1) as singles,
            tc.tile_pool(name="stats", bufs=4) as stats,
            tc.tile_pool(name="t2p", bufs=3) as t2p,
            tc.tile_pool(name="t2tp", bufs=6) as t2tp,
            tc.tile_pool(name="psum_tt", bufs=4, space="PSUM") as psum_tt,
            tc.tile_pool(name="psum_mm", bufs=2, space="PSUM") as psum_mm,
            tc.tile_pool(name="atoms", bufs=4) as atoms_pool,
            tc.tile_pool(name="gath", bufs=4) as gath_pool,
        ):
            # --- constants / small inputs ---
            idx_sb = singles.tile([P, N_ATOMS // 16], mybir.dt.int16)
            nc.sync.dma_start(out=idx_sb[:], in_=idx16.ap())
            wg_sb = singles.tile([P, C_S], f32)
            nc.sync.dma_start(out=wg_sb[:], in_=wg.ap())
            wt_sb = singles.tile([P, C_S], f32)
            nc.sync.dma_start(out=wt_sb[:], in_=wt.ap())
            beta_sb = singles.tile([P, KC], f32)
            nc.sync.dma_start(out=beta_sb[:], in_=beta3.ap())

            ident = singles.tile([P, P], f32)
            make_identity(nc, ident[:])
            ones1 = singles.tile([1, P], f32)
            nc.vector.memset(ones1[:], 1.0)
            eps_t = singles.tile([P, 1], f32)
            nc.vector.memset(eps_t[:], EPS)

            # --- bias row: (beta @ W.T) [1, C_ATOM] ---
            bias_ps = psum_mm.tile([1, C_ATOM], f32, tag="biasps")
            for k in range(KC):
                nc.tensor.matmul(
                    bias_ps[:],
                    lhsT=beta_sb[:, k : k + 1],
                    rhs=wt_sb[:, k * P : (k + 1) * P],
                    start=(k == 0),
                    stop=(k == KC - 1),
                )
            bias_sb = singles.tile([1, C_ATOM], f32)
            nc.vector.tensor_copy(out=bias_sb[:], in_=bias_ps[:])
            # broadcast bias across partitions once: ones[1,P].T @ bias[1,P]
            bias_bc_ps = psum_mm.tile([P, C_ATOM], f32, tag="updps")
            nc.tensor.matmul(
                bias_bc_ps[:], lhsT=ones1[:], rhs=bias_sb[:], start=True, stop=True
            )
            bias_bc = singles.tile([P, C_ATOM], f32)
            nc.vector.tensor_copy(out=bias_bc[:], in_=bias_bc_ps[:])

            prev_gathers = []
            for _rep in range(repeat):
                # --- load s p-major: s_big[p, j, :] = row p*16 + j, so the
                # load (and the matching upd store) is contiguous per
                # partition. LN is per-row, so any row->lane mapping works.
                # Split into quarters so LN on tile 0 starts ~4x sooner.
                s_big = singles.tile([P, N_RES_TILES, C_S], f32, tag="s_big")
                s_pt = s_in.ap().rearrange("(p t) c -> p t c", p=P)
                for sq in range(4):
                    tq = N_RES_TILES // 4
                    nc.sync.dma_start(
                        out=s_big[:, sq * tq : (sq + 1) * tq, :],
                        in_=s_pt[:, sq * tq : (sq + 1) * tq, :],
                    )

                # --- LN + matmul into upd table ---
                upd_big = singles.tile([P, N_RES_TILES, C_ATOM], f32, tag="upd_big")
                for i in range(N_RES_TILES):
                    st6 = stats.tile([P, 6], f32)
                    nc.vector.bn_stats(out=st6[:], in_=s_big[:, i, :])
                    mv = stats.tile([P, 2], f32)
                    nc.vector.bn_aggr(out=mv[:], in_=st6[:])
                    std = stats.tile([P, 1], f32)
                    nc.scalar.activation(
                        out=std[:],
                        in_=mv[:, 1:2],
                        func=mybir.ActivationFunctionType.Sqrt,
                        bias=eps_t[:],
                    )
                    rstd = stats.tile([P, 1], f32)
                    nc.vector.reciprocal(out=rstd[:], in_=std[:])
                    t2 = t2p.tile([P, C_S], f32)
                    nc.vector.tensor_scalar(
                        out=t2[:],
                        in0=s_big[:, i, :],
                        scalar1=mv[:, 0:1],
                        scalar2=rstd[:],
                        op0=mybir.AluOpType.subtract,
                        op1=mybir.AluOpType.mult,
                    )
                    t2t_chunks = []
                    for k in range(KC):
                        tt_ps = psum_tt.tile([P, P], f32)
                        nc.tensor.transpose(
                            out=tt_ps[:],
                            in_=t2[:, k * P : (k + 1) * P],
                            identity=ident[:],
                        )
                        t2t = t2tp.tile([P, P], f32)
                        nc.vector.tensor_copy(out=t2t[:], in_=tt_ps[:])
                        t2t_chunks.append(t2t)
                    upd_ps = psum_mm.tile([P, C_ATOM], f32, tag="updps")
                    for k in range(KC):
                        nc.tensor.matmul(
                            upd_ps[:],
                            lhsT=t2t_chunks[k][:],
                            rhs=wg_sb[:, k * P : (k + 1) * P],
                            start=(k == 0),
                            stop=(k == KC - 1),
                        )
                    # fold the beta bias in during the PSUM->SBUF move (DVE)
                    # instead of a 4th K=1 matmul: keeps PE on the critical
                    # path free for transposes/matmuls
                    nc.vector.tensor_tensor(
                        out=upd_big[:, i, :],
                        in0=upd_ps[:],
                        in1=bias_bc[:],
                        op=mybir.AluOpType.add,
                    )

                # store table in quarters so stores overlap later LN tiles;
                # gathers gate on all four. p-major: upd_big[p, j, :] is
                # table row p*16 + j -> contiguous per partition.
                upd_stores = []
                upd_pt = upd_dram.ap().rearrange("(p t) c -> p t c", p=P)
                for sq in range(4):
                    tq = N_RES_TILES // 4
                    us = nc.sync.dma_start(
                        out=upd_pt[:, sq * tq : (sq + 1) * tq, :],
                        in_=upd_big[:, sq * tq : (sq + 1) * tq, :],
                    )
                    upd_stores.append(us)
                # WAR: don't overwrite the table while last rep still gathers
                for pg in prev_gathers:
                    for us in upd_stores:
                        add_dep_helper(
                            us.ins, pg.ins, reason="WAR on upd table across reps"
                        )
                prev_gathers = []

                # --- gather + residual add over 8 chunks of 4096 atoms ---
                for c in range(N_CHUNKS):
                    at = atoms_pool.tile([P, CT, C_ATOM], f32, tag="at")
                    nc.sync.dma_start(
                        out=at[:],
                        in_=atom.ap()[c * CHUNK : (c + 1) * CHUNK, :].rearrange(
                            "(p t) c -> p t c", p=P
                        ),
                    )
                    g = gath_pool.tile([P, CT, C_ATOM], f32, tag="g")
                    for q in range(GQ):
                        gi = nc.gpsimd.dma_gather(
                            g[:, q * GT : (q + 1) * GT, :],
                            upd_dram.ap(),
                            idx_sb[
                                :,
                                c * (CHUNK // 16) + q * (GSUB // 16) : c * (CHUNK // 16)
                                + (q + 1) * (GSUB // 16),
                            ],
                            GSUB,
                            GSUB,
                            C_ATOM,
                            queue_num=(c * GQ + q) % NQUEUES,
                        )
                        for us in upd_stores:
                            add_dep_helper(
                                gi.ins, us.ins, reason="upd table must be in DRAM"
                            )
                        prev_gathers.append(gi)
                    nc.vector.tensor_add(out=at[:], in0=at[:], in1=g[:])
                    nc.sync.dma_start(
                        out=out.ap()[c * CHUNK : (c + 1) * CHUNK, :].rearrange(
                            "(p t) c -> p t c", p=P
                        ),
                        in_=at[:],
                    )

    nc.compile()
    return nc


def _prep_core_inputs(atom_embed, s, cond_to_s_idx, ln_gamma, ln_beta, W):
    """Host-side sharding + layout marshalling (no math beyond folding the
    LN scale into the weight layout)."""
    # gamma-folded W^T, chunked so cs-chunk k sits at free columns [k*128, ...)
    wg_full = (W * ln_gamma[None, :]).T.astype(np.float32)  # [C_S, C_ATOM]
    wg_host = np.ascontiguousarray(
        wg_full.reshape(KC, P, C_ATOM).transpose(1, 0, 2).reshape(P, C_S)
    )
    wt_full = np.ascontiguousarray(W.T.astype(np.float32))  # [C_S, C_ATOM]
    wt_host = np.ascontiguousarray(
        wt_full.reshape(KC, P, C_ATOM).transpose(1, 0, 2).reshape(P, C_S)
    )
    beta_host = np.ascontiguousarray(
        ln_beta.astype(np.float32).reshape(KC, P).T
    )  # [P, KC]

    in_maps = []
    for b in range(B):
        idxb = cond_to_s_idx[b].astype(np.int16)  # values < 2048
        # atom tile layout within a 4096-chunk: at[p, t] = atom p*CT + t.
        # sub-gather q writes g[j%128, q*GT + j//128] for list position j, so
        # position j of sub-gather q must hold the index of atom
        # (j%128)*CT + q*GT + j//128.
        A = idxb.reshape(N_CHUNKS, P, GQ, GT)  # [c, p, q, u]
        L = A.transpose(0, 2, 3, 1).reshape(N_CHUNKS, GQ, GSUB)  # j = u*128+p
        # wrap within each sub-gather: position j -> [j%16, j//16]
        Wr = L.reshape(N_CHUNKS, GQ, GSUB // 16, 16).transpose(0, 1, 3, 2)
        idx_full = np.ascontiguousarray(
            Wr.transpose(2, 0, 1, 3).reshape(16, N_ATOMS // 16)
        )
        idx_rep = np.ascontiguousarray(np.tile(idx_full, (P // 16, 1)))
        in_maps.append(
            {
                "atom": np.ascontiguousarray(atom_embed[b]),
                "s_in": np.ascontiguousarray(s[b]),
                "idx16": idx_rep,
                "wg": wg_host,
                "wt": wt_host,
                "beta3": beta_host,
            }
        )
    return in_maps


def kernel(atom_embed, s, cond_to_s_idx, ln_gamma, ln_beta, W):
    global _compiled
    from concourse.bass_utils import run_bass_kernel_spmd

    atom_embed = np.asarray(atom_embed, dtype=np.float32)
    s = np.asarray(s, dtype=np.float32)
    cond_to_s_idx = np.asarray(cond_to_s_idx)
    ln_gamma = np.asarray(ln_gamma, dtype=np.float32)
    ln_beta = np.asarray(ln_beta, dtype=np.float32)
    W = np.asarray(W, dtype=np.float32)

    if _compiled is None:
        _compiled = _build()
    in_maps = _prep_core_inputs(atom_embed, s, cond_to_s_idx, ln_gamma, ln_beta, W)
    res = run_bass_kernel_spmd(_compiled, in_maps, core_ids=list(range(B)))
    out = np.stack([res.results[b]["out"] for b in range(B)], axis=0)
    return out


# revision 27
# speedup vs baseline: 1.3837x; 1.0864x over previous
"""Bass/Trainium2 kernel for nn_GatherUpdate: LayerNorm + Linear + per-atom
row gather + residual add, data-parallel over batch across 8 NeuronCores.

reference:
    normed = LayerNorm(s) * gamma + beta            # s: [B, 2048, 384]
    upd    = normed @ W.T                           # W: [128, 384] -> [B, 2048, 128]
    out    = atom_embed + upd[:, cond_to_s_idx, :]  # atom_embed: [B, 32768, 128]

Per-core plan (core b handles batch b):
  1. LN over 16 tiles of [128 res, 384] via bn_stats/bn_aggr.
  2. PE-transpose normed tiles (cs onto partitions), matmul against
     gamma-folded W^T chunks, accumulate in PSUM; beta folded in via a K=1
     ones-row matmul against (beta @ W.T).
  3. Write upd table [2048, 128] to DRAM scratch.
  4. dma_gather 512B rows from the table (32 calls x 1024 idxs — the HW
     SWDGE ring holds ~127 descriptors/engine — rotated over 4 SWDGE
     queues), add to atom_embed tiles, store. Atom tiles use contiguous
     4MB DMAs; the host pre-permutes the int16 index stream so the gather
     output layout matches the atom tiles.
"""

import sys

sys.path.insert(0, "/opt/trn_rl_repo")

import numpy as np

B = 8
N_ATOMS = 32768
N_RES = 2048
C_S = 384
C_ATOM = 128
EPS = 1e-5
P = 128
N_RES_TILES = N_RES // P  # 16
N_CHUNKS = 4  # atom chunks per core
CHUNK = N_ATOMS // N_CHUNKS  # 8192 atoms
CT = CHUNK // P  # 64 tiles of 128 atoms per chunk
KC = C_S // P  # 3 contraction chunks
GSUB = 1024  # idxs per dma_gather (HW SWDGE ring holds ~127 desc/engine)
GQ = CHUNK // GSUB  # 8 sub-gathers per atom chunk
GT = GSUB // P  # 8 x 128-atom tiles per sub-gather
NQUEUES = 4  # rotate SWDGE queues so desc-gen overlaps in-flight gathers

_compiled = None


def _build(repeat=1):
    """Build the per-core program. repeat>1 unrolls the whole pipeline N
    times (used only for timing: wall(N)-wall(1) cancels dispatch/transfer
    overhead)."""
    import concourse.bacc as bacc
    import concourse.tile as tile
    from concourse import mybir
    from concourse.masks import make_identity
    from concourse.tile import add_dep_helper

    f32 = mybir.dt.float32

    nc = bacc.Bacc(
        "TRN2", target_bir_lowering=False, debug=False, num_swdge_queues=NQUEUES
    )

    atom = nc.dram_tensor("atom", [N_ATOMS, C_ATOM], f32, kind="ExternalInput")
    s_in = nc.dram_tensor("s_in", [N_RES, C_S], f32, kind="ExternalInput")
    idx16 = nc.dram_tensor(
        "idx16", [P, N_ATOMS // 16], mybir.dt.int16, kind="ExternalInput"
    )
    wg = nc.dram_tensor("wg", [P, C_S], f32, kind="ExternalInput")
    wt = nc.dram_tensor("wt", [P, C_S], f32, kind="ExternalInput")
    beta3 = nc.dram_tensor("beta3", [P, KC], f32, kind="ExternalInput")
    out = nc.dram_tensor("out", [N_ATOMS, C_ATOM], f32, kind="ExternalOutput")
    upd_dram = nc.dram_tensor("upd_scratch", [N_RES, C_ATOM], f32, kind="Internal")

    with tile.TileContext(nc) as tc:
        with (
            tc.tile_pool(name="singles", bufs=1) as singles,
            tc.tile_pool(name="stats", bufs=4) as stats,
            tc.tile_pool(name="t2p", bufs=3) as t2p,
            tc.tile_pool(name="t2tp", bufs=6) as t2tp,
            tc.tile_pool(name="psum_tt", bufs=4, space="PSUM") as psum_tt,
            tc.tile_pool(name="psum_mm", bufs=2, space="PSUM") as psum_mm,
            tc.tile_pool(name="atoms", bufs=2) as atoms_pool,
            tc.tile_pool(name="gath", bufs=2) as gath_pool,
        ):
            # --- constants / small inputs ---
            idx_sb = singles.tile([P, N_ATOMS // 16], mybir.dt.int16)
            nc.sync.dma_start(out=idx_sb[:], in_=idx16.ap())
            wg_sb = singles.tile([P, C_S], f32)
            nc.sync.dma_start(out=wg_sb[:], in_=wg.ap())
            wt_sb = singles.tile([P, C_S], f32)
            nc.sync.dma_start(out=wt_sb[:], in_=wt.ap())
            beta_sb = singles.tile([P, KC], f32)
            nc.sync.dma_start(out=beta_sb[:], in_=beta3.ap())

            ident = singles.tile([P, P], f32)
            make_identity(nc, ident[:])
            ones1 = singles.tile([1, P], f32)
            nc.vector.memset(ones1[:], 1.0)
            eps_t = singles.tile([P, 1], f32)
            nc.vector.memset(eps_t[:], EPS)

            # --- bias row: (beta @ W.T) [1, C_ATOM] ---
            bias_ps = psum_mm.tile([1, C_ATOM], f32, tag="biasps")
            for k in range(KC):
                nc.tensor.matmul(
                    bias_ps[:],
                    lhsT=beta_sb[:, k : k + 1],
                    rhs=wt_sb[:, k * P : (k + 1) * P],
                    start=(k == 0),
                    stop=(k == KC - 1),
                )
            bias_sb = singles.tile([1, C_ATOM], f32)
            nc.vector.tensor_copy(out=bias_sb[:], in_=bias_ps[:])
            # broadcast bias across partitions once: ones[1,P].T @ bias[1,P]
            bias_bc_ps = psum_mm.tile([P, C_ATOM], f32, tag="updps")
            nc.tensor.matmul(
                bias_bc_ps[:], lhsT=ones1[:], rhs=bias_sb[:], start=True, stop=True
            )
            bias_bc = singles.tile([P, C_ATOM], f32)
            nc.vector.tensor_copy(out=bias_bc[:], in_=bias_bc_ps[:])

            prev_gathers = []
            for _rep in range(repeat):
                # --- load s p-major: s_big[p, j, :] = row p*16 + j, so the
                # load (and the matching upd store) is contiguous per
                # partition. LN is per-row, so any row->lane mapping works.
                # Split into quarters so LN on tile 0 starts ~4x sooner.
                s_big = singles.tile([P, N_RES_TILES, C_S], f32, tag="s_big")
                s_pt = s_in.ap().rearrange("(p t) c -> p t c", p=P)
                for sq in range(4):
                    tq = N_RES_TILES // 4
                    nc.sync.dma_start(
                        out=s_big[:, sq * tq : (sq + 1) * tq, :],
                        in_=s_pt[:, sq * tq : (sq + 1) * tq, :],
                    )

                # --- LN + matmul into upd table ---
                upd_big = singles.tile([P, N_RES_TILES, C_ATOM], f32, tag="upd_big")
                for i in range(N_RES_TILES):
                    st6 = stats.tile([P, 6], f32)
                    nc.vector.bn_stats(out=st6[:], in_=s_big[:, i, :])
                    mv = stats.tile([P, 2], f32)
                    nc.vector.bn_aggr(out=mv[:], in_=st6[:])
                    std = stats.tile([P, 1], f32)
                    nc.scalar.activation(
                        out=std[:],
                        in_=mv[:, 1:2],
                        func=mybir.ActivationFunctionType.Sqrt,
                        bias=eps_t[:],
                    )
                    rstd = stats.tile([P, 1], f32)
                    nc.vector.reciprocal(out=rstd[:], in_=std[:])
                    t2 = t2p.tile([P, C_S], f32)
                    nc.vector.tensor_scalar(
                        out=t2[:],
                        in0=s_big[:, i, :],
                        scalar1=mv[:, 0:1],
                        scalar2=rstd[:],
                        op0=mybir.AluOpType.subtract,
                        op1=mybir.AluOpType.mult,
                    )
                    t2t_chunks = []
                    for k in range(KC):
                        tt_ps = psum_tt.tile([P, P], f32)
                        nc.tensor.transpose(
                            out=tt_ps[:],
                            in_=t2[:, k * P : (k + 1) * P],
                            identity=ident[:],
                        )
                        t2t = t2tp.tile([P, P], f32)
                        nc.vector.tensor_copy(out=t2t[:], in_=tt_ps[:])
                        t2t_chunks.append(t2t)
                    upd_ps = psum_mm.tile([P, C_ATOM], f32, tag="updps")
                    for k in range(KC):
                        nc.tensor.matmul(
                            upd_ps[:],
                            lhsT=t2t_chunks[k][:],
                            rhs=wg_sb[:, k * P : (k + 1) * P],
                            start=(k == 0),
                            stop=(k == KC - 1),
                        )
                    # fold the beta bias in during the PSUM->SBUF move (DVE)
                    # instead of a 4th K=1 matmul: keeps PE on the critical
                    # path free for transposes/matmuls
                    nc.vector.tensor_tensor(
                        out=upd_big[:, i, :],
                        in0=upd_ps[:],
                        in1=bias_bc[:],
                        op=mybir.AluOpType.add,
                    )

                # store table in quarters so stores overlap later LN tiles;
                # gathers gate on all four. p-major: upd_big[p, j, :] is
                # table row p*16 + j -> contiguous per partition.
                upd_stores = []
                upd_pt = upd_dram.ap().rearrange("(p t) c -> p t c", p=P)
                for sq in range(4):
                    tq = N_RES_TILES // 4
                    us = nc.sync.dma_start(
                        out=upd_pt[:, sq * tq : (sq + 1) * tq, :],
                        in_=upd_big[:, sq * tq : (sq + 1) * tq, :],
                    )
                    upd_stores.append(us)
                # WAR: don't overwrite the table while last rep still gathers
                for pg in prev_gathers:
                    for us in upd_stores:
                        add_dep_helper(
                            us.ins, pg.ins, reason="WAR on upd table across reps"
                        )
                prev_gathers = []

                # --- gather + residual add over 4 chunks of 8192 atoms ---
                for c in range(N_CHUNKS):
                    at = atoms_pool.tile([P, CT, C_ATOM], f32, tag="at")
                    nc.sync.dma_start(
                        out=at[:],
                        in_=atom.ap()[c * CHUNK : (c + 1) * CHUNK, :].rearrange(
                            "(p t) c -> p t c", p=P
                        ),
                    )
                    g = gath_pool.tile([P, CT, C_ATOM], f32, tag="g")
                    for q in range(GQ):
                        gi = nc.gpsimd.dma_gather(
                            g[:, q * GT : (q + 1) * GT, :],
                            upd_dram.ap(),
                            idx_sb[
                                :,
                                c * (CHUNK // 16) + q * (GSUB // 16) : c * (CHUNK // 16)
                                + (q + 1) * (GSUB // 16),
                            ],
                            GSUB,
                            GSUB,
                            C_ATOM,
                            queue_num=(c * GQ + q) % NQUEUES,
                        )
                        for us in upd_stores:
                            add_dep_helper(
                                gi.ins, us.ins, reason="upd table must be in DRAM"
                            )
                        prev_gathers.append(gi)
                    nc.vector.tensor_add(out=at[:], in0=at[:], in1=g[:])
                    nc.sync.dma_start(
                        out=out.ap()[c * CHUNK : (c + 1) * CHUNK, :].rearrange(
                            "(p t) c -> p t c", p=P
                        ),
                        in_=at[:],
                    )

    nc.compile()
    return nc


def _prep_core_inputs(atom_embed, s, cond_to_s_idx, ln_gamma, ln_beta, W):
    """Host-side sharding + layout marshalling (no math beyond folding the
    LN scale into the weight layout)."""
    # gamma-folded W^T, chunked so cs-chunk k sits at free columns [k*128, ...)
    wg_full = (W * ln_gamma[None, :]).T.astype(np.float32)  # [C_S, C_ATOM]
    wg_host = np.ascontiguousarray(
        wg_full.reshape(KC, P, C_ATOM).transpose(1, 0, 2).reshape(P, C_S)
    )
    wt_full = np.ascontiguousarray(W.T.astype(np.float32))  # [C_S, C_ATOM]
    wt_host = np.ascontiguousarray(
        wt_full.reshape(KC, P, C_ATOM).transpose(1, 0, 2).reshape(P, C_S)
    )
    beta_host = np.ascontiguousarray(
        ln_beta.astype(np.float32).reshape(KC, P).T
    )  # [P, KC]

    in_maps = []
    for b in range(B):
        idxb = cond_to_s_idx[b].astype(np.int16)  # values < 2048
        # atom tile layout within a 4096-chunk: at[p, t] = atom p*CT + t.
        # sub-gather q writes g[j%128, q*GT + j//128] for list position j, so
        # position j of sub-gather q must hold the index of atom
        # (j%128)*CT + q*GT + j//128.
        A = idxb.reshape(N_CHUNKS, P, GQ, GT)  # [c, p, q, u]
        L = A.transpose(0, 2, 3, 1).reshape(N_CHUNKS, GQ, GSUB)  # j = u*128+p
        # wrap within each sub-gather: position j -> [j%16, j//16]
        Wr = L.reshape(N_CHUNKS, GQ, GSUB // 16, 16).transpose(0, 1, 3, 2)
        idx_full = np.ascontiguousarray(
            Wr.transpose(2, 0, 1, 3).reshape(16, N_ATOMS // 16)
        )
        idx_rep = np.ascontiguousarray(np.tile(idx_full, (P // 16, 1)))
        in_maps.append(
            {
                "atom": np.ascontiguousarray(atom_embed[b]),
                "s_in": np.ascontiguousarray(s[b]),
                "idx16": idx_rep,
                "wg": wg_host,
                "wt": wt_host,
                "beta3": beta_host,
            }
        )
    return in_maps


def kernel(atom_embed, s, cond_to_s_idx, ln_gamma, ln_beta, W):
    global _compiled
    from concourse.bass_utils import run_bass_kernel_spmd

    atom_embed = np.asarray(atom_embed, dtype=np.float32)
    s = np.asarray(s, dtype=np.float32)
    cond_to_s_idx = np.asarray(cond_to_s_idx)
    ln_gamma = np.asarray(ln_gamma, dtype=np.float32)
    ln_beta = np.asarray(ln_beta, dtype=np.float32)
    W = np.asarray(W, dtype=np.float32)

    if _compiled is None:
        _compiled = _build()
    in_maps = _prep_core_inputs(atom_embed, s, cond_to_s_idx, ln_gamma, ln_beta, W)
    res = run_bass_kernel_spmd(_compiled, in_maps, core_ids=list(range(B)))
    out = np.stack([res.results[b]["out"] for b in range(B)], axis=0)
    return out
